# revision 1
# baseline (speedup 1.0000x reference)
"""Trainium2 Bass kernel for nn_Couple_loss_62380105007762.

Loss = w0 * MSE + w1 * KLD + w2 * CE where
  sig(x)  = 2 * x[:, 0].sum(axis=F)                      (inverse SSQ-STFT, real channel only)
  MSE     = sum((sig(output_rec) - sig(target_rec))**2)
  KLD     = -0.5 * sum(1 + log_var - mean**2 - exp(log_var))
  CE      = mean cross-entropy(output_clas, target_clas)

Sharding: data-parallel over the batch dim (64 rows -> 8 cores x 8 rows).
Each core computes a weighted partial loss scalar; host sums the 8 partials
(the "psum" of per-shard losses).

Device strategy per core (memory-bound problem; only the REAL channel of the
rec tensors is ever read -> 2 x 8 MB of f32 traffic per core):
  - For each of the 8 batch rows: DMA the [F=128, T=2048] real-channel plane
    of output_rec and target_rec (1 MB contiguous each).
  - Partition-dim reduction over F via TensorE ones-matmul: psum[1, T] =
    (+1s)^T @ o + (-1s)^T @ t accumulated in PSUM (float32r mode, full-rate).
  - Square-and-accumulate psum[1, T] -> scalar on DVE/ACT (alternating).
  - KLD/CE terms on the tiny [8, 256]/[8, 5] shards with fused
    activation-accumulate ops.
  - Final weighted combine via one fused multiply-reduce against
    host-prepared effective weights; scalar DMA'd out.
"""

import numpy as np
from contextlib import ExitStack

import concourse.bass as bass
import concourse.tile as tile
from concourse import mybir
from concourse.bass_utils import run_bass_kernel_spmd

N_CORES = 8
B, Z, F, T, C = 64, 256, 128, 2048, 5
BS = B // N_CORES  # batch rows per core
N_CHUNK = 512      # matmul moving-operand max free dim (fp32)

FP32 = mybir.dt.float32
FP32R = mybir.dt.float32r
AX = mybir.AxisListType
ALU = mybir.AluOpType
ACTF = mybir.ActivationFunctionType


def build_bass(legalize: bool = True):
    nc = bass.Bass()

    # float32r: same 32-bit data, lets the PE run matmuls at full rate
    # (fp32 matmul is 4 cycles/row; fp32r with free dim >= 256 is 1).
    o_rec = nc.declare_dram_parameter("o_rec", [BS, F, T], FP32R, isOutput=False)
    t_rec = nc.declare_dram_parameter("t_rec", [BS, F, T], FP32R, isOutput=False)
    mean_in = nc.declare_dram_parameter("mean_in", [BS, Z], FP32, isOutput=False)
    logvar_in = nc.declare_dram_parameter("logvar_in", [BS, Z], FP32, isOutput=False)
    oclas = nc.declare_dram_parameter("oclas", [BS, C], FP32, isOutput=False)
    onehot = nc.declare_dram_parameter("onehot", [BS, C], FP32, isOutput=False)
    # w_eff = [4*w0, -0.5*w1, w2/64, -1024*w1] (host-prepared)
    w_eff = nc.declare_dram_parameter("w_eff", [1, 4], FP32, isOutput=False)
    # +1/-1 matmul weight columns; shipped as data because DVE may not
    # memset a float32r tile (walrus ISA check)
    pm = nc.declare_dram_parameter("pm", [F, 2], FP32R, isOutput=False)
    out = nc.declare_dram_parameter("out", [1, 1], FP32, isOutput=True)

    with tile.TileContext(nc) as tc:
        with ExitStack() as ctx:
            const_pool = ctx.enter_context(tc.tile_pool(name="const", bufs=1))
            o_pool = ctx.enter_context(tc.tile_pool(name="opool", bufs=4))
            t_pool = ctx.enter_context(tc.tile_pool(name="tpool", bufs=4))
            # PSUM budget (8 banks): ps [1,T]=4 banks x bufs=1, plus 1 bank
            # for the kc reduction.
            ps_pool = ctx.enter_context(tc.tile_pool(name="ps", bufs=1, space="PSUM"))
            pskc_pool = ctx.enter_context(tc.tile_pool(name="pskc", bufs=1, space="PSUM"))
            junk_pool = ctx.enter_context(tc.tile_pool(name="junk", bufs=2))
            small = ctx.enter_context(tc.tile_pool(name="small", bufs=1))

            pm_t = const_pool.tile([F, 2], FP32R, tag="pm")
            nc.gpsimd.dma_start(pm_t[:], pm[:, :])
            ones = pm_t[:, 0:1]
            nones = pm_t[:, 1:2]

            # ---- small terms (KLD / CE) on their tiny shards ----
            m_t = small.tile([BS, Z], FP32, tag="m")
            lv_t = small.tile([BS, Z], FP32, tag="lv")
            oc_t = small.tile([BS, C], FP32, tag="oc")
            oh_t = small.tile([BS, C], FP32, tag="oh")
            w_t = small.tile([1, 4], FP32, tag="w")
            nc.gpsimd.dma_start(m_t[:], mean_in[:, :])
            nc.gpsimd.dma_start(lv_t[:], logvar_in[:, :])
            nc.gpsimd.dma_start(oc_t[:], oclas[:, :])
            nc.gpsimd.dma_start(oh_t[:], onehot[:, :])
            nc.gpsimd.dma_start(w_t[:], w_eff[:, :])

            # KLD rows: kld_row[b] = sum_z(log_var) - sum_z(mean^2) - sum_z(exp(log_var))
            msq_sum = small.tile([BS, 1], FP32, tag="msq")
            e_sum = small.tile([BS, 1], FP32, tag="esum")
            lv_sum = small.tile([BS, 1], FP32, tag="lvsum")
            kl_junk = small.tile([BS, Z], FP32, tag="klj")
            kl_junk2 = small.tile([BS, Z], FP32, tag="klj2")
            nc.vector.tensor_tensor(kl_junk[:], m_t[:], m_t[:], ALU.mult)
            nc.vector.reduce_sum(msq_sum[:], kl_junk[:], axis=AX.X)
            nc.scalar.activation(kl_junk2[:], lv_t[:], ACTF.Exp, accum_out=e_sum[:])
            nc.vector.reduce_sum(lv_sum[:], lv_t[:], axis=AX.X)

            # kc[:, 0] = kld_row, kc[:, 1] = ce_row
            kc = small.tile([BS, 2], FP32, tag="kc")
            kl_tmp = small.tile([BS, 1], FP32, tag="kltmp")
            nc.vector.tensor_tensor(kl_tmp[:], lv_sum[:], msq_sum[:], ALU.subtract)
            nc.vector.tensor_tensor(kc[:, 0:1], kl_tmp[:], e_sum[:], ALU.subtract)

            # CE rows: ce_row[b] = rowmax + log(sum(exp(oc - rowmax))) - oc[b, y_b]
            rowmax = small.tile([BS, 1], FP32, tag="rmax")
            nmax = small.tile([BS, 1], FP32, tag="nmax")
            sumexp = small.tile([BS, 1], FP32, tag="sexp")
            lse = small.tile([BS, 1], FP32, tag="lse")
            picked = small.tile([BS, 1], FP32, tag="picked")
            ce_junk = small.tile([BS, C], FP32, tag="cej")
            ce_junk2 = small.tile([BS, C], FP32, tag="cej2")
            ce_tmp = small.tile([BS, 1], FP32, tag="cetmp")
            nc.vector.reduce_max(rowmax[:], oc_t[:], axis=AX.X)
            nc.vector.tensor_scalar_mul(nmax[:], rowmax[:], -1.0)
            nc.scalar.activation(
                ce_junk[:], oc_t[:], ACTF.Exp, bias=nmax[:], accum_out=sumexp[:]
            )
            nc.scalar.activation(lse[:], sumexp[:], ACTF.Ln)
            nc.vector.tensor_tensor(ce_junk2[:], oc_t[:], oh_t[:], ALU.mult)
            nc.vector.reduce_sum(picked[:], ce_junk2[:], axis=AX.X)
            nc.vector.tensor_tensor(ce_tmp[:], rowmax[:], lse[:], ALU.add)
            nc.vector.tensor_tensor(kc[:, 1:2], ce_tmp[:], picked[:], ALU.subtract)

            # ---- main MSE stream ----
            sq_acc = const_pool.tile([1, BS], FP32, tag="sqacc")
            for b in range(BS):
                o_tile = o_pool.tile([F, T], FP32R, tag="o")
                t_tile = t_pool.tile([F, T], FP32R, tag="t")
                nc.sync.dma_start(o_tile[:], o_rec[b, :, :])
                nc.scalar.dma_start(t_tile[:], t_rec[b, :, :])

                ps = ps_pool.tile([1, T], FP32, tag="ps")
                for k in range(T // N_CHUNK):
                    sl = slice(k * N_CHUNK, (k + 1) * N_CHUNK)
                    nc.tensor.matmul(
                        ps[:, sl], ones, o_tile[:, sl], start=True, stop=False
                    )
                    prev_last_mm = nc.tensor.matmul(
                        ps[:, sl], nones, t_tile[:, sl], start=False, stop=True
                    )
                # square + accumulate sum over T on ACT (only one PSUM input
                # allowed per instruction, so DVE ps*ps is illegal)
                junk = junk_pool.tile([1, T], FP32, tag="junk")
                nc.scalar.activation(
                    junk[:], ps[:], ACTF.Square,
                    accum_out=sq_acc[0:1, b:b + 1],
                )

            # partition-sum of kc[8, 2] via ones-matmul -> psum [1, 2]
            ones_bs = const_pool.tile([BS, 1], FP32, tag="onesbs")
            nc.vector.memset(ones_bs[:], 1.0)
            ps_kc = pskc_pool.tile([1, 2], FP32, tag="pskc")
            nc.tensor.matmul(ps_kc[:], ones_bs[:], kc[:], start=True, stop=True)

            # v = [mse_S, kld_S, ce_S, 1.0]; result = dot(v, w_eff)
            v = small.tile([1, 4], FP32, tag="v")
            vjunk = small.tile([1, 4], FP32, tag="vjunk")
            res = small.tile([1, 1], FP32, tag="res")
            nc.vector.reduce_sum(v[0:1, 0:1], sq_acc[:], axis=AX.X)
            nc.vector.tensor_copy(v[0:1, 1:3], ps_kc[:])
            nc.vector.memset(v[0:1, 3:4], 1.0)
            nc.vector.tensor_tensor(vjunk[:], v[:], w_t[:], ALU.mult)
            nc.vector.reduce_sum(res[:], vjunk[:], axis=AX.X)
            nc.sync.dma_start(out[:, :], res[:])

    if legalize:
        # CoreSim's race detector rejects the hoisted wait instructions
        # (no Tile fake sem updates), so sim runs build with legalize=False.
        _legalize_multi_waits(nc)
    # Populate .instr bytes for extended-ISA instructions
    # (tensor_tensor_reduce) — raw Bass skips Bacc's lowering pass and the
    # NEFF compiler fails with "ISA wrong length" without this.
    mybir.codegen_inst_isa_subclasses(nc)
    return nc


def _legalize_multi_waits(nc):
    """walrus rejects TPB compute instructions carrying more than one sync
    wait ("Too many sync wait commands" in the S3 encodings — hit for both
    Matmult/S3_LW and Activation/S3D3_AC). Hoist every wait of a multi-wait
    compute instruction onto standalone InstEventSemaphore instructions
    (exactly what `engine.wait_ge()` emits) inserted just before it on the
    same engine. DMA instructions keep their waits (DGE path handles many).
    """
    for fn in nc.m.functions:
        for blk in fn.blocks:
            new_insts = []
            for inst in blk.instructions:
                si = inst.sync_info
                tname = type(inst).__name__
                if (
                    si is not None
                    and si.on_wait
                    and len(si.on_wait) > 1
                    and tname != "InstEventSemaphore"
                ):
                    for i, w in enumerate(si.on_wait):
                        new_insts.append(
                            mybir.InstEventSemaphore(
                                name=f"{inst.name}_hoistw{i}",
                                engine=inst.engine,
                                ins=[],
                                outs=[],
                                sync_info=mybir.SyncInfo(on_wait=[w], on_update=[]),
                            )
                        )
                    inst.sync_info = mybir.SyncInfo(
                        on_wait=[], on_update=si.on_update
                    )
                new_insts.append(inst)
            blk.instructions = new_insts


_NC_CACHE = {}


def _get_nc():
    if "nc" not in _NC_CACHE:
        _NC_CACHE["nc"] = build_bass()
    return _NC_CACHE["nc"]


def make_in_maps(inputs) -> list[dict]:
    o = np.asarray(inputs["output_rec"], dtype=np.float32)
    t = np.asarray(inputs["target_rec"], dtype=np.float32)
    mean = np.asarray(inputs["mean"], dtype=np.float32)
    log_var = np.asarray(inputs["log_var"], dtype=np.float32)
    oclas = np.asarray(inputs["output_clas"], dtype=np.float32)
    tclas = np.asarray(inputs["target_clas"]).astype(np.int64)
    w = np.asarray(inputs["weight"], dtype=np.float32).astype(np.float64)

    # Only the real channel contributes to the inverse SSQ-STFT.
    o_real = np.ascontiguousarray(o[:, 0])  # [B, F, T]
    t_real = np.ascontiguousarray(t[:, 0])

    onehot = np.zeros((B, C), dtype=np.float32)
    onehot[np.arange(B), tclas] = 1.0

    # Effective weights folding ISSQ_SCALE^2=4 (MSE), -0.5 and the
    # sum-of-ones constant (KLD: per-core 8*256=2048 ones), 1/B (CE mean).
    w_eff = np.array(
        [[4.0 * w[0], -0.5 * w[1], w[2] / B, -0.5 * w[1] * (BS * Z)]],
        dtype=np.float32,
    )
    pm = np.stack(
        [np.ones(F, dtype=np.float32), -np.ones(F, dtype=np.float32)], axis=1
    )

    in_maps = []
    for c in range(N_CORES):
        s = slice(c * BS, (c + 1) * BS)
        in_maps.append(
            {
                "o_rec": o_real[s],
                "t_rec": t_real[s],
                "mean_in": mean[s],
                "logvar_in": log_var[s],
                "oclas": oclas[s],
                "onehot": onehot[s],
                "w_eff": w_eff,
                "pm": pm,
            }
        )
    return in_maps


def kernel(**inputs) -> np.ndarray:
    in_maps = make_in_maps(inputs)
    nc = _get_nc()
    res = run_bass_kernel_spmd(nc, in_maps, list(range(N_CORES)))
    total = sum(float(r["out"][0, 0]) for r in res.results)
    return np.float32(total)



# revision 2
# speedup vs baseline: 2.0728x; 2.0728x over previous
"""Trainium2 Bass kernel for nn_Couple_loss_62380105007762.

Loss = w0 * MSE + w1 * KLD + w2 * CE where
  sig(x)  = 2 * x[:, 0].sum(axis=F)                      (inverse SSQ-STFT, real channel only)
  MSE     = sum((sig(output_rec) - sig(target_rec))**2)
  KLD     = -0.5 * sum(1 + log_var - mean**2 - exp(log_var))
  CE      = mean cross-entropy(output_clas, target_clas)

Sharding: data-parallel over the batch dim (64 rows -> 8 cores x 8 rows).
Each core computes a weighted partial loss scalar; host sums the 8 partials.

Device strategy (memory-bound problem): ship the real channels as fp8 e4m3
(rel-err contribution ~1e-3, gate is 2e-2), 4 MiB per core instead of 16.
  - DRAM layout is the flat-block view [128, 8, 2048]: partition p holds
    16 KB contiguous (batch row p//16, f-planes 8*(p%16)..+8), so DMA runs
    at line rate. o on the sync HWDGE queue, t on the scalar HWDGE queue,
    4 x 512 KB pieces each, so both queues stream concurrently (~HBM rate).
  - The host negates target_rec before fp8 conversion, so the f-reduction
    of o and -t accumulates diff = sig_o - sig_t directly in PSUM.
  - DoubleRow fp8 matmuls (2 k-planes per pass) with a block-selector
    stationary: chunk k of t maps to psum partitions [8k, 8k+8), giving one
    [32, 512] PSUM tile = diff[b, t] for all 8 batch rows (1 bank).
  - ACT square + accumulate -> per-partition MSE partials; ones-matmul
    partition-reduce; fused weighted combine against host-prepared w_eff.
  - KLD/CE computed from one packed [8, 532] f32 side tensor on DVE/ACT
    while the main stream DMAs.
"""

import numpy as np
import ml_dtypes
from contextlib import ExitStack

import concourse.bass as bass
import concourse.tile as tile
from concourse import mybir
from concourse.bass_utils import run_bass_kernel_spmd

N_CORES = 8
B, Z, F, T, C = 64, 256, 128, 2048, 5
BS = B // N_CORES   # batch rows per core
NJ = 8              # f-planes per partition line (flat-block layout)
NCHUNK = 4          # t-chunks of 512
CW = T // NCHUNK    # 512 columns per chunk
N_WARM = 10         # PE warm-up matmuls (HAM un-throttle)

FP32 = mybir.dt.float32
FP8 = mybir.dt.float8e4
AX = mybir.AxisListType
ALU = mybir.AluOpType
ACTF = mybir.ActivationFunctionType
DR = mybir.MatmulPerfMode.DoubleRow

# packed [8, 532] f32 side-tensor column map
SM_MEAN = slice(0, 256)
SM_LV = slice(256, 512)
SM_OC = slice(512, 517)
SM_OH = slice(517, 522)
SM_W = slice(522, 526)


def build_bass(legalize: bool = True):
    nc = bass.Bass()

    o8 = nc.declare_dram_parameter("o8", [128, NJ, T], FP8, isOutput=False)
    t8 = nc.declare_dram_parameter("t8", [128, NJ, T], FP8, isOutput=False)
    sw = nc.declare_dram_parameter("sw", [128, 2, 128], FP8, isOutput=False)
    small = nc.declare_dram_parameter("small", [BS, 532], FP32, isOutput=False)
    out = nc.declare_dram_parameter("out", [1, 1], FP32, isOutput=True)

    with tile.TileContext(nc) as tc:
        with ExitStack() as ctx:
            const_pool = ctx.enter_context(tc.tile_pool(name="const", bufs=1))
            big_pool = ctx.enter_context(tc.tile_pool(name="big", bufs=1))
            small_pool = ctx.enter_context(tc.tile_pool(name="small", bufs=1))
            ps_pool = ctx.enter_context(tc.tile_pool(name="ps", bufs=1, space="PSUM"))
            psw_pool = ctx.enter_context(tc.tile_pool(name="psw", bufs=1, space="PSUM"))
            psv_pool = ctx.enter_context(tc.tile_pool(name="psv", bufs=1, space="PSUM"))

            sw_t = const_pool.tile([128, 2, 128], FP8, tag="sw")
            small_t = small_pool.tile([BS, 532], FP32, tag="small")
            nc.gpsimd.dma_start(sw_t[:], sw[:, :, :])
            nc.gpsimd.dma_start(small_t[:], small[:, :])

            o_t = big_pool.tile([128, NJ, T], FP8, tag="o")
            t_t = big_pool.tile([128, NJ, T], FP8, tag="t")
            for i in range(NJ // 2):
                js = slice(2 * i, 2 * i + 2)
                nc.sync.dma_start(o_t[:, js, :], o8[:, js, :])
                nc.scalar.dma_start(t_t[:, js, :], t8[:, js, :])

            # scratch32: col0 = MSE row partials (ACT accum), col1 = KLD rows,
            # col2 = CE rows, col3 = [1, 0, ...] for the constant KLD term.
            scratch32 = small_pool.tile([32, 4], FP32, tag="sc32")
            nc.vector.memset(scratch32[:], 0.0)
            nc.vector.memset(scratch32[0:1, 3:4], 1.0)
            ones32 = small_pool.tile([32, 1], FP32, tag="ones32")
            nc.vector.memset(ones32[:], 1.0)

            # ---- KLD / CE on the packed side tensor (overlaps main DMA) ----
            m_t = small_t[:, SM_MEAN]
            lv_t = small_t[:, SM_LV]
            oc_t = small_t[:, SM_OC]
            oh_t = small_t[:, SM_OH]

            msq = small_pool.tile([BS, 1], FP32, tag="msq")
            esum = small_pool.tile([BS, 1], FP32, tag="esum")
            lvsum = small_pool.tile([BS, 1], FP32, tag="lvsum")
            kl_j = small_pool.tile([BS, Z], FP32, tag="klj")
            kl_j2 = small_pool.tile([BS, Z], FP32, tag="klj2")
            kl_tmp = small_pool.tile([BS, 1], FP32, tag="kltmp")
            nc.vector.tensor_tensor(kl_j[:], m_t, m_t, ALU.mult)
            nc.vector.reduce_sum(msq[:], kl_j[:], axis=AX.X)
            nc.scalar.activation(kl_j2[:], lv_t, ACTF.Exp, accum_out=esum[:])
            nc.vector.reduce_sum(lvsum[:], lv_t, axis=AX.X)
            nc.vector.tensor_tensor(kl_tmp[:], lvsum[:], msq[:], ALU.subtract)
            nc.vector.tensor_tensor(
                scratch32[0:BS, 1:2], kl_tmp[:], esum[:], ALU.subtract
            )

            rmax = small_pool.tile([BS, 1], FP32, tag="rmax")
            nmax = small_pool.tile([BS, 1], FP32, tag="nmax")
            sexp = small_pool.tile([BS, 1], FP32, tag="sexp")
            lse = small_pool.tile([BS, 1], FP32, tag="lse")
            picked = small_pool.tile([BS, 1], FP32, tag="picked")
            ce_j = small_pool.tile([BS, C], FP32, tag="cej")
            ce_j2 = small_pool.tile([BS, C], FP32, tag="cej2")
            ce_tmp = small_pool.tile([BS, 1], FP32, tag="cetmp")
            nc.vector.reduce_max(rmax[:], oc_t, axis=AX.X)
            nc.vector.tensor_scalar_mul(nmax[:], rmax[:], -1.0)
            nc.scalar.activation(
                ce_j[:], oc_t, ACTF.Exp, bias=nmax[:], accum_out=sexp[:]
            )
            nc.scalar.activation(lse[:], sexp[:], ACTF.Ln)
            nc.vector.tensor_tensor(ce_j2[:], oc_t, oh_t, ALU.mult)
            nc.vector.reduce_sum(picked[:], ce_j2[:], axis=AX.X)
            nc.vector.tensor_tensor(ce_tmp[:], rmax[:], lse[:], ALU.add)
            nc.vector.tensor_tensor(
                scratch32[0:BS, 2:3], ce_tmp[:], picked[:], ALU.subtract
            )

            # ---- PE warm-up: junk DoubleRow matmuls on sw to lift HAM ----
            ps_w = psw_pool.tile([128, 128], FP32, tag="psw")
            for _ in range(N_WARM):
                nc.tensor.matmul(
                    ps_w[:], sw_t[:, :, :], sw_t[:, :, :],
                    start=True, stop=True, perf_mode=DR,
                )

            # ---- main MSE stream ----
            # ps[8k + b, c] accumulates diff[b, 512k + c]; every matmul writes
            # the full [32, CW] region (selector zeros elsewhere), so only the
            # very first carries start=True.
            ps = ps_pool.tile([32, CW], FP32, tag="ps")
            n_mm = (NJ // 2) * 2 * NCHUNK
            mm = 0
            for i in range(NJ // 2):
                js = slice(2 * i, 2 * i + 2)
                for tens in (o_t, t_t):
                    for k in range(NCHUNK):
                        nc.tensor.matmul(
                            ps[:],
                            sw_t[:, :, 32 * k:32 * k + 32],
                            tens[:, js, CW * k:CW * k + CW],
                            start=(mm == 0),
                            stop=(mm == n_mm - 1),
                            perf_mode=DR,
                        )
                        mm += 1

            # ---- epilogue ----
            junk = small_pool.tile([32, CW], FP32, tag="junk")
            nc.scalar.activation(
                junk[:], ps[:], ACTF.Square, accum_out=scratch32[:, 0:1]
            )
            ps_v = psv_pool.tile([1, 4], FP32, tag="psv")
            nc.tensor.matmul(
                ps_v[:], ones32[:], scratch32[:], start=True, stop=True
            )
            vj = small_pool.tile([1, 4], FP32, tag="vj")
            res = small_pool.tile([1, 1], FP32, tag="res")
            nc.vector.tensor_tensor(vj[:], ps_v[:], small_t[0:1, SM_W], ALU.mult)
            nc.vector.reduce_sum(res[:], vj[:], axis=AX.X)
            nc.sync.dma_start(out[:, :], res[:])

    if legalize:
        _legalize_multi_waits(nc)
    mybir.codegen_inst_isa_subclasses(nc)
    return nc


def _legalize_multi_waits(nc):
    """walrus rejects TPB compute instructions carrying more than one sync
    wait. Hoist every wait of a multi-wait compute instruction onto
    standalone InstEventSemaphore instructions on the same engine."""
    for fn in nc.m.functions:
        for blk in fn.blocks:
            new_insts = []
            for inst in blk.instructions:
                si = inst.sync_info
                tname = type(inst).__name__
                if (
                    si is not None
                    and si.on_wait
                    and len(si.on_wait) > 1
                    and tname != "InstEventSemaphore"
                ):
                    for i, w in enumerate(si.on_wait):
                        new_insts.append(
                            mybir.InstEventSemaphore(
                                name=f"{inst.name}_hoistw{i}",
                                engine=inst.engine,
                                ins=[],
                                outs=[],
                                sync_info=mybir.SyncInfo(on_wait=[w], on_update=[]),
                            )
                        )
                    inst.sync_info = mybir.SyncInfo(
                        on_wait=[], on_update=si.on_update
                    )
                new_insts.append(inst)
            blk.instructions = new_insts


_NC_CACHE = {}


def _get_nc():
    if "nc" not in _NC_CACHE:
        _NC_CACHE["nc"] = build_bass()
    return _NC_CACHE["nc"]


def make_in_maps(inputs) -> list[dict]:
    o = np.asarray(inputs["output_rec"], dtype=np.float32)
    t = np.asarray(inputs["target_rec"], dtype=np.float32)
    mean = np.asarray(inputs["mean"], dtype=np.float32)
    log_var = np.asarray(inputs["log_var"], dtype=np.float32)
    oclas = np.asarray(inputs["output_clas"], dtype=np.float32)
    tclas = np.asarray(inputs["target_clas"]).astype(np.int64)
    w = np.asarray(inputs["weight"], dtype=np.float32).astype(np.float64)

    # Only the real channel contributes; negate target so the PE accumulates
    # sig_o - sig_t directly. fp8 e4m3 keeps the loss well inside the 2e-2
    # gate (measured ~9e-4).
    o8 = o[:, 0].astype(ml_dtypes.float8_e4m3)          # [B, F, T]
    t8 = np.negative(t[:, 0]).astype(ml_dtypes.float8_e4m3)

    onehot = np.zeros((B, C), dtype=np.float32)
    onehot[np.arange(B), tclas] = 1.0

    # [4*w0 (ISSQ scale^2), -w1/2, w2/B, -w1/2 * (BS*Z) const-term]
    w_eff = np.array(
        [4.0 * w[0], -0.5 * w[1], w[2] / B, -0.5 * w[1] * (BS * Z)],
        dtype=np.float32,
    )

    # DoubleRow block-selector stationary: chunk k (columns [32k, 32k+32))
    # routes batch row b = p//16 to psum partition 8k + b, same weight for
    # both k-planes of the pair.
    sw_np = np.zeros((128, 2, 128), dtype=ml_dtypes.float8_e4m3)
    p = np.arange(128)
    for k in range(NCHUNK):
        sw_np[p, :, 40 * k + p // 16] = 1.0

    in_maps = []
    for c in range(N_CORES):
        s = slice(c * BS, (c + 1) * BS)
        small_np = np.zeros((BS, 532), dtype=np.float32)
        small_np[:, SM_MEAN] = mean[s]
        small_np[:, SM_LV] = log_var[s]
        small_np[:, SM_OC] = oclas[s]
        small_np[:, SM_OH] = onehot[s]
        small_np[0, SM_W] = w_eff
        in_maps.append(
            {
                # [8, 128, 2048] -> flat-block [128, 8, 2048]: partition
                # p = b*16 + f//8 holds 16 KB contiguous DRAM.
                "o8": o8[s].reshape(128, NJ, T),
                "t8": t8[s].reshape(128, NJ, T),
                "sw": sw_np,
                "small": small_np,
            }
        )
    return in_maps


def kernel(**inputs) -> np.ndarray:
    in_maps = make_in_maps(inputs)
    nc = _get_nc()
    res = run_bass_kernel_spmd(nc, in_maps, list(range(N_CORES)))
    total = sum(float(r["out"][0, 0]) for r in res.results)
    return np.float32(total)


# revision 3
# speedup vs baseline: 2.6496x; 1.2783x over previous
"""Trainium2 Bass kernel for nn_Couple_loss_62380105007762.

Loss = w0 * MSE + w1 * KLD + w2 * CE where
  sig(x)  = 2 * x[:, 0].sum(axis=F)                      (inverse SSQ-STFT, real channel only)
  MSE     = sum((sig(output_rec) - sig(target_rec))**2)
  KLD     = -0.5 * sum(1 + log_var - mean**2 - exp(log_var))
  CE      = mean cross-entropy(output_clas, target_clas)

Sharding: data-parallel over the batch dim (64 rows -> 8 cores x 8 rows).
Each core computes a weighted partial loss scalar; host sums the 8 partials
(plus the data-independent KLD constant).

Device strategy (memory-bound problem): ship the real channels as fp8 e4m3
(loss rel-err ~9e-4, gate is 2e-2), 4 MiB per core instead of 16.
  - DRAM layout is the flat-block view [128, 8, 2048]: partition p holds
    16 KB contiguous DRAM (batch row p//16, f-planes 8*(p%16)..+8), so DMA
    runs with large line-contiguous descriptors. o streams on the sync
    HWDGE queue, t on the scalar HWDGE queue, 2 x 1 MiB pieces each.
  - The host negates target_rec before fp8 conversion, so accumulating
    both tensors under the same +1 selector yields diff = sig_o - sig_t.
  - Plain fp8 matmuls, 4x column-tiled: t-chunk k -> PE column group k
    (tile_position (0, 32k)), so 4 matmuls run concurrently and PSUM
    collects diff[b, t] as [128, 512] (rows 32k + b) in a single bank.
  - ACT square + accumulate -> per-partition MSE partials -> ones-matmul
    partition reduce -> weighted dot with host-prepared w_eff -> DMA out.
  - KLD/CE computed from one packed [8, 532] f32 side tensor on DVE/ACT
    while the main stream DMAs; PE warm-up matmuls lift the HAM throttle
    before the data arrives.
"""

import numpy as np
import ml_dtypes
from contextlib import ExitStack

import concourse.bass as bass
import concourse.tile as tile
from concourse import mybir
from concourse.bass_utils import run_bass_kernel_spmd

N_CORES = 8
B, Z, F, T, C = 64, 256, 128, 2048, 5
BS = B // N_CORES   # batch rows per core
NJ = 8              # f-planes per partition line (flat-block layout)
NCHUNK = 4          # t-chunks of 512 -> 4 PE column groups
CW = T // NCHUNK    # 512 columns per chunk
NPIECE = 2          # DMA pieces per tensor (1 MiB each)
JP = NJ // NPIECE   # f-planes per piece
N_WARM = 18         # PE warm-up matmuls (HAM un-throttle)

FP32 = mybir.dt.float32
FP8 = mybir.dt.float8e4
AX = mybir.AxisListType
ALU = mybir.AluOpType
ACTF = mybir.ActivationFunctionType

# packed [8, 532] f32 side-tensor column map
SM_MEAN = slice(0, 256)
SM_LV = slice(256, 512)
SM_OC = slice(512, 517)
SM_OH = slice(517, 522)
SM_W = slice(522, 525)


def build_bass(legalize: bool = True):
    nc = bass.Bass()

    o8 = nc.declare_dram_parameter("o8", [128, NJ, T], FP8, isOutput=False)
    t8 = nc.declare_dram_parameter("t8", [128, NJ, T], FP8, isOutput=False)
    sw = nc.declare_dram_parameter("sw", [128, 32], FP8, isOutput=False)
    small = nc.declare_dram_parameter("small", [BS, 532], FP32, isOutput=False)
    out = nc.declare_dram_parameter("out", [1, 1], FP32, isOutput=True)

    with tile.TileContext(nc) as tc:
        with ExitStack() as ctx:
            const_pool = ctx.enter_context(tc.tile_pool(name="const", bufs=1))
            big_pool = ctx.enter_context(tc.tile_pool(name="big", bufs=1))
            small_pool = ctx.enter_context(tc.tile_pool(name="small", bufs=1))
            ps_pool = ctx.enter_context(tc.tile_pool(name="ps", bufs=1, space="PSUM"))
            psw_pool = ctx.enter_context(tc.tile_pool(name="psw", bufs=1, space="PSUM"))
            psv_pool = ctx.enter_context(tc.tile_pool(name="psv", bufs=1, space="PSUM"))

            # constants ride the fast HWDGE queues ahead of the big pieces
            sw_t = const_pool.tile([128, 32], FP8, tag="sw")
            small_t = small_pool.tile([BS, 532], FP32, tag="small")
            nc.sync.dma_start(sw_t[:], sw[:, :])
            nc.scalar.dma_start(small_t[:], small[:, :])

            o_t = big_pool.tile([128, NJ, T], FP8, tag="o")
            t_t = big_pool.tile([128, NJ, T], FP8, tag="t")
            for i in range(NPIECE):
                js = slice(JP * i, JP * (i + 1))
                nc.sync.dma_start(o_t[:, js, :], o8[:, js, :])
                nc.scalar.dma_start(t_t[:, js, :], t8[:, js, :])

            # scratch: col0 = MSE row partials (ACT accum), col1 = KLD rows,
            # col2 = CE rows.
            scratch = small_pool.tile([128, 3], FP32, tag="scr")
            nc.vector.memset(scratch[:], 0.0)
            ones128 = small_pool.tile([128, 1], FP32, tag="ones")
            nc.vector.memset(ones128[:], 1.0)

            # ---- KLD / CE on the packed side tensor (overlaps main DMA) ----
            m_t = small_t[:, SM_MEAN]
            lv_t = small_t[:, SM_LV]
            oc_t = small_t[:, SM_OC]
            oh_t = small_t[:, SM_OH]

            msq = small_pool.tile([BS, 1], FP32, tag="msq")
            esum = small_pool.tile([BS, 1], FP32, tag="esum")
            lvsum = small_pool.tile([BS, 1], FP32, tag="lvsum")
            kl_j = small_pool.tile([BS, Z], FP32, tag="klj")
            kl_j2 = small_pool.tile([BS, Z], FP32, tag="klj2")
            kl_tmp = small_pool.tile([BS, 1], FP32, tag="kltmp")
            nc.vector.tensor_tensor(kl_j[:], m_t, m_t, ALU.mult)
            nc.vector.reduce_sum(msq[:], kl_j[:], axis=AX.X)
            nc.scalar.activation(kl_j2[:], lv_t, ACTF.Exp, accum_out=esum[:])
            nc.vector.reduce_sum(lvsum[:], lv_t, axis=AX.X)
            nc.vector.tensor_tensor(kl_tmp[:], lvsum[:], msq[:], ALU.subtract)
            nc.vector.tensor_tensor(
                scratch[0:BS, 1:2], kl_tmp[:], esum[:], ALU.subtract
            )

            rmax = small_pool.tile([BS, 1], FP32, tag="rmax")
            nmax = small_pool.tile([BS, 1], FP32, tag="nmax")
            sexp = small_pool.tile([BS, 1], FP32, tag="sexp")
            lse = small_pool.tile([BS, 1], FP32, tag="lse")
            picked = small_pool.tile([BS, 1], FP32, tag="picked")
            ce_j = small_pool.tile([BS, C], FP32, tag="cej")
            ce_j2 = small_pool.tile([BS, C], FP32, tag="cej2")
            ce_tmp = small_pool.tile([BS, 1], FP32, tag="cetmp")
            nc.vector.reduce_max(rmax[:], oc_t, axis=AX.X)
            nc.vector.tensor_scalar_mul(nmax[:], rmax[:], -1.0)
            nc.scalar.activation(
                ce_j[:], oc_t, ACTF.Exp, bias=nmax[:], accum_out=sexp[:]
            )
            nc.scalar.activation(lse[:], sexp[:], ACTF.Ln)
            nc.vector.tensor_tensor(ce_j2[:], oc_t, oh_t, ALU.mult)
            nc.vector.reduce_sum(picked[:], ce_j2[:], axis=AX.X)
            nc.vector.tensor_tensor(ce_tmp[:], rmax[:], lse[:], ALU.add)
            nc.vector.tensor_tensor(
                scratch[0:BS, 2:3], ce_tmp[:], picked[:], ALU.subtract
            )

            # ---- PE warm-up on the tiny selector tile (lift HAM early) ----
            ps_w = psw_pool.tile([32, 32], FP32, tag="psw")
            for _ in range(N_WARM):
                nc.tensor.matmul(ps_w[:], sw_t[:], sw_t[:], start=True, stop=True)

            # ---- main MSE stream: plain fp8, 4x column-tiled ----
            # ps[32k + b, c] accumulates diff[b, 512k + c]; column group k
            # runs concurrently with the others. The selector writes all 32
            # rows of its group (zeros beyond row 8).
            ps = ps_pool.tile([128, CW], FP32, tag="ps")
            groups = [(i, tens) for i in range(NPIECE) for tens in ("o", "t")]
            tiles = {"o": o_t, "t": t_t}
            for gi, (i, tname) in enumerate(groups):
                tens = tiles[tname]
                for j in range(JP * i, JP * (i + 1)):
                    for k in range(NCHUNK):
                        first = gi == 0 and j == JP * i
                        last = gi == len(groups) - 1 and j == JP * (i + 1) - 1
                        nc.tensor.matmul(
                            ps[32 * k:32 * k + 32, :],
                            sw_t[:],
                            tens[:, j, CW * k:CW * k + CW],
                            start=first,
                            stop=last,
                            tile_position=(0, 32 * k),
                            skip_group_check=True,
                        )

            # ---- epilogue ----
            junk = small_pool.tile([128, CW], FP32, tag="junk")
            nc.scalar.activation(
                junk[:], ps[:], ACTF.Square, accum_out=scratch[:, 0:1]
            )
            ps_v = psv_pool.tile([1, 3], FP32, tag="psv")
            nc.tensor.matmul(
                ps_v[:], ones128[:], scratch[:], start=True, stop=True
            )
            vj = small_pool.tile([1, 3], FP32, tag="vj")
            res = small_pool.tile([1, 1], FP32, tag="res")
            nc.vector.tensor_tensor(vj[:], ps_v[:], small_t[0:1, SM_W], ALU.mult)
            nc.vector.reduce_sum(res[:], vj[:], axis=AX.X)
            nc.sync.dma_start(out[:, :], res[:])

    if legalize:
        _legalize_multi_waits(nc)
    mybir.codegen_inst_isa_subclasses(nc)
    return nc


def _legalize_multi_waits(nc):
    """walrus rejects TPB compute instructions carrying more than one sync
    wait. Hoist every wait of a multi-wait compute instruction onto
    standalone InstEventSemaphore instructions on the same engine."""
    for fn in nc.m.functions:
        for blk in fn.blocks:
            new_insts = []
            for inst in blk.instructions:
                si = inst.sync_info
                tname = type(inst).__name__
                if (
                    si is not None
                    and si.on_wait
                    and len(si.on_wait) > 1
                    and tname != "InstEventSemaphore"
                ):
                    for i, w in enumerate(si.on_wait):
                        new_insts.append(
                            mybir.InstEventSemaphore(
                                name=f"{inst.name}_hoistw{i}",
                                engine=inst.engine,
                                ins=[],
                                outs=[],
                                sync_info=mybir.SyncInfo(on_wait=[w], on_update=[]),
                            )
                        )
                    inst.sync_info = mybir.SyncInfo(
                        on_wait=[], on_update=si.on_update
                    )
                new_insts.append(inst)
            blk.instructions = new_insts


_NC_CACHE = {}


def _get_nc():
    if "nc" not in _NC_CACHE:
        _NC_CACHE["nc"] = build_bass()
    return _NC_CACHE["nc"]


def make_in_maps(inputs) -> list[dict]:
    o = np.asarray(inputs["output_rec"], dtype=np.float32)
    t = np.asarray(inputs["target_rec"], dtype=np.float32)
    mean = np.asarray(inputs["mean"], dtype=np.float32)
    log_var = np.asarray(inputs["log_var"], dtype=np.float32)
    oclas = np.asarray(inputs["output_clas"], dtype=np.float32)
    tclas = np.asarray(inputs["target_clas"]).astype(np.int64)
    w = np.asarray(inputs["weight"], dtype=np.float32).astype(np.float64)

    # Only the real channel contributes; negate target so the PE accumulates
    # sig_o - sig_t directly under one +1 selector.
    o8 = o[:, 0].astype(ml_dtypes.float8_e4m3)          # [B, F, T]
    t8 = np.negative(t[:, 0]).astype(ml_dtypes.float8_e4m3)

    onehot = np.zeros((B, C), dtype=np.float32)
    onehot[np.arange(B), tclas] = 1.0

    # [4*w0 (ISSQ scale^2), -w1/2, w2/B]; the data-independent KLD constant
    # -w1/2 * B * Z is added on the host.
    w_eff = np.array([4.0 * w[0], -0.5 * w[1], w[2] / B], dtype=np.float32)

    # selector: batch row b = p//16 -> column b (columns 8..32 zero)
    sw_np = np.zeros((128, 32), dtype=ml_dtypes.float8_e4m3)
    p = np.arange(128)
    sw_np[p, p // 16] = 1.0

    in_maps = []
    for c in range(N_CORES):
        s = slice(c * BS, (c + 1) * BS)
        small_np = np.zeros((BS, 532), dtype=np.float32)
        small_np[:, SM_MEAN] = mean[s]
        small_np[:, SM_LV] = log_var[s]
        small_np[:, SM_OC] = oclas[s]
        small_np[:, SM_OH] = onehot[s]
        small_np[0, SM_W] = w_eff
        in_maps.append(
            {
                # [8, 128, 2048] -> flat-block [128, 8, 2048]: partition
                # p = b*16 + f//8 holds 16 KB contiguous DRAM.
                "o8": o8[s].reshape(128, NJ, T),
                "t8": t8[s].reshape(128, NJ, T),
                "sw": sw_np,
                "small": small_np,
            }
        )
    return in_maps


def kernel(**inputs) -> np.ndarray:
    in_maps = make_in_maps(inputs)
    nc = _get_nc()
    res = run_bass_kernel_spmd(nc, in_maps, list(range(N_CORES)))
    w1 = float(np.asarray(inputs["weight"], dtype=np.float64)[1])
    total = sum(float(r["out"][0, 0]) for r in res.results)
    total += -0.5 * w1 * B * Z  # data-independent KLD constant
    return np.float32(total)
